# revision 3
# baseline (speedup 1.0000x reference)
"""Bass/Trainium2 kernel for batched kNN-interpolate + MSE (nn_KnnMSE).

Reference computation (see problem):
  d2[i,j] = ||c2_i - c1_j||^2, masked to same-graph pairs (b1/b2 sorted),
  top-k=8 smallest per target row, w = 1/clip(d2, 1e-16),
  interp = sum(w * f1[idx]) / sum(w),  out = mean((interp - f2)^2).

Strategy: b1/b2 are SORTED graph ids => the valid distance matrix is
block-diagonal over the 64 graphs.  Each graph is an independent
[~256 targets, ~256 sources] kNN problem.  We pad each graph to a fixed
shape (S=320 source slots, 3x128 target rows), assign 8 graphs to each
of the 8 NeuronCores, and on-device per 128-target chunk:

  1. PE    : dist matmul  psum = 2*c2.c1 - ||c1||^2   (K=4 augmented)
  2. ACT   : d2 = relu(-psum + ||c2||^2)              (clamped >= 0)
  3. DVE   : rec = 1/d2  (d2 >= 2.7e-4 for this data; never 0)
  4. DVE   : top8 = max8(rec); thresh = top8[:,7]  (8 nearest = largest rec)
  5. DVE   : W = (rec >= thresh) * rec, sumw = rowsum(W)   [one fused op]
  6. PE    : Wt = W^T (3x 128x128 transposes via identity)
  7. PE    : numer = Wt^T.T @ f1  (contract over sources, PSUM-accumulated)
  8. DVE   : err = numer * (mask/sumw) - f2     (padded rows -> 0)
  9. ACT   : acc[:, chunk] = sum_d(err^2)
  finally : per-partition totals -> DRAM; host sums 8x128 values / (N*D).

Padded sources get coords=1e4 (d2 ~ 3e8, never in top-8, weight->0 via
mask).  Padded targets get coords=0 (finite everywhere) and are zeroed
in step 8 by the mask folded into 1/sumw.

Self-contained: hardcodes shapes from the problem spec; computes graph
boundaries from the actual b1/b2 at call time (host-side index prep only).
"""

import numpy as np

# Problem constants
N = 16384
D = 128
B = 64
KNN = 8
NCORES = 8
GPC = B // NCORES        # graphs per core
S = 320                  # padded source slots per graph (max count 292)
SCH = 3                  # 128-row source chunks covering S (ceil(320/128))
SPAD = SCH * 128         # 384 source slots in the chunked layout
TCH = 3                  # 128-row target chunks per graph (max count 292)
TPAD = TCH * 128
BIGC = 1.0e4             # padded source coordinate


def _build_nc():
    import concourse.bacc as bacc
    import concourse.mybir as mybir
    import concourse.tile as tile
    from concourse.masks import make_identity

    f32 = mybir.dt.float32
    AF = mybir.ActivationFunctionType
    OP = mybir.AluOpType

    nc = bacc.Bacc("TRN2", target_bir_lowering=False, debug=False)

    c1r_d = nc.dram_tensor("c1r", [GPC, 4, S], f32, kind="ExternalInput")
    f1a_d = nc.dram_tensor("f1a", [GPC, 128, SCH, D], f32, kind="ExternalInput")
    c2t_d = nc.dram_tensor("c2t", [GPC, 4, TCH, 128], f32, kind="ExternalInput")
    cm_d = nc.dram_tensor("cm", [GPC, 128, TCH, 2], f32, kind="ExternalInput")
    f2_d = nc.dram_tensor("f2", [GPC, 128, TCH, D], f32, kind="ExternalInput")
    out_d = nc.dram_tensor("out_sums", [128, 1], f32, kind="ExternalOutput")

    with tile.TileContext(nc) as tc:
        with (
            tc.tile_pool(name="constp", bufs=1) as constp,
            tc.tile_pool(name="gbuf", bufs=2) as gbuf,
            tc.tile_pool(name="work", bufs=3) as work,
            tc.tile_pool(name="small", bufs=4) as small,
            tc.tile_pool(name="pdp", bufs=2, space="PSUM") as pdp,
            tc.tile_pool(name="ptp", bufs=3, space="PSUM") as ptp,
            tc.tile_pool(name="pip", bufs=2, space="PSUM") as pip_,
        ):
            ident = constp.tile([128, 128], f32)
            make_identity(nc, ident)
            acc = constp.tile([128, GPC * TCH], f32)
            nc.vector.memset(acc, 0.0)

            for g in range(GPC):
                c1r_t = gbuf.tile([4, S], f32, tag="c1r")
                nc.sync.dma_start(c1r_t, c1r_d[g])
                f1a_t = gbuf.tile([128, SCH, D], f32, tag="f1a")
                nc.sync.dma_start(f1a_t, f1a_d[g])
                c2t_t = gbuf.tile([4, TCH, 128], f32, tag="c2t")
                nc.sync.dma_start(c2t_t, c2t_d[g])
                cm_t = gbuf.tile([128, TCH, 2], f32, tag="cm")
                nc.sync.dma_start(cm_t, cm_d[g])
                f2_t = gbuf.tile([128, TCH, D], f32, tag="f2")
                nc.sync.dma_start(f2_t, f2_d[g])

                for t in range(TCH):
                    ci = g * TCH + t
                    # 1) distances: psum = 2*c2.c1 - ||c1||^2  [128 targets, S]
                    pd = pdp.tile([128, S], f32, tag="pd")
                    nc.tensor.matmul(pd, c2t_t[:, t], c1r_t, start=True, stop=True)
                    # 2) d2 = relu(-psum + ||c2||^2)
                    d2 = work.tile([128, S], f32, tag="d2")
                    nc.scalar.activation(
                        d2, pd, AF.Relu, bias=cm_t[:, t, 0:1], scale=-1.0
                    )
                    # 3) rec = 1/d2
                    rec = work.tile([128, S], f32, tag="rec")
                    nc.vector.reciprocal(rec, d2)
                    # 4) 8 largest reciprocals per row (= 8 nearest)
                    top8 = small.tile([128, 8], f32, tag="top8")
                    nc.vector.max(out=top8, in_=rec)
                    # 5) W = (rec >= thresh) * rec ; sumw = rowsum(W)
                    W = work.tile([128, S], f32, tag="W")
                    sumw = small.tile([128, 1], f32, tag="sumw")
                    nc.vector.scalar_tensor_tensor(
                        out=W,
                        in0=rec,
                        scalar=top8[:, 7:8],
                        in1=rec,
                        op0=OP.is_ge,
                        op1=OP.mult,
                        accum_out=sumw,
                    )
                    # 6) Wt = W^T (PE transpose per 128-col block)
                    wt = work.tile([128, SCH, 128], f32, tag="wt")
                    for k in range(SCH):
                        w0 = 128 * k
                        cw = min(S, w0 + 128) - w0
                        pt = ptp.tile([128, 128], f32, tag="pt")
                        nc.tensor.transpose(pt[:cw], W[:, w0 : w0 + cw], ident)
                        if k < 2:
                            nc.scalar.copy(wt[:cw, k], pt[:cw])
                        else:
                            nc.vector.tensor_copy(wt[:cw, k], pt[:cw])
                        if cw < 128:
                            # stale SBUF here would feed the PE matmul
                            # (NaN * 0 = NaN); zero the unused rows
                            nc.gpsimd.memset(wt[cw:, k], 0.0)
                    # rsw = mask / sumw
                    rsw = small.tile([128, 1], f32, tag="rsw")
                    nc.vector.reciprocal(rsw, sumw)
                    rswm = small.tile([128, 1], f32, tag="rswm")
                    nc.vector.tensor_scalar(
                        rswm, rsw, cm_t[:, t, 1:2], None, op0=OP.mult
                    )
                    # 7) numer = sum_k Wt[:,k]^T @ f1a[:,k]   [128 targets, D]
                    pi = pip_.tile([128, D], f32, tag="pi")
                    for k in range(SCH):
                        nc.tensor.matmul(
                            pi,
                            wt[:, k],
                            f1a_t[:, k],
                            start=(k == 0),
                            stop=(k == SCH - 1),
                        )
                    # 8) err = numer * rswm - f2  (padded target rows -> 0)
                    err = work.tile([128, D], f32, tag="err")
                    nc.vector.scalar_tensor_tensor(
                        out=err,
                        in0=pi,
                        scalar=rswm,
                        in1=f2_t[:, t],
                        op0=OP.mult,
                        op1=OP.subtract,
                    )
                    # 9) acc[:, ci] = sum_d err^2
                    sq = work.tile([128, D], f32, tag="sq")
                    nc.scalar.activation(
                        sq, err, AF.Square, accum_out=acc[:, ci : ci + 1]
                    )

            tot = constp.tile([128, 1], f32)
            nc.vector.reduce_sum(tot, acc, axis=mybir.AxisListType.X)
            nc.sync.dma_start(out_d[:, :], tot)

    nc.compile()
    return nc


def _prep_in_maps(inputs):
    x1 = np.ascontiguousarray(np.asarray(inputs["x1"], dtype=np.float32))
    x2 = np.ascontiguousarray(np.asarray(inputs["x2"], dtype=np.float32))
    b1 = np.asarray(inputs["b1"]).astype(np.int64)
    b2 = np.asarray(inputs["b2"]).astype(np.int64)

    c1, f1 = x1[:, :3], x1[:, 3:]
    c2, f2 = x2[:, :3], x2[:, 3:]

    gs = np.arange(B + 1)
    e1 = np.searchsorted(b1, gs)
    e2 = np.searchsorted(b2, gs)
    n1 = np.diff(e1)
    n2 = np.diff(e2)
    assert n1.max() <= S, f"source count {n1.max()} exceeds padded S={S}"
    assert n2.max() <= TPAD, f"target count {n2.max()} exceeds padded T={TPAD}"
    assert n1.min() >= KNN, f"graph with fewer than {KNN} sources"

    c1r = np.empty((B, 4, S), np.float32)
    f1a = np.zeros((B, SPAD, D), np.float32)
    c2t = np.zeros((B, 4, TCH, 128), np.float32)
    cm = np.zeros((B, 128, TCH, 2), np.float32)
    f2p = np.zeros((B, 128, TCH, D), np.float32)

    for g in range(B):
        a, bb = e1[g], e1[g + 1]
        n = n1[g]
        cc = np.full((S, 3), BIGC, np.float32)
        cc[:n] = c1[a:bb]
        c1r[g, :3] = 2.0 * cc.T
        c1r[g, 3] = -np.einsum("ij,ij->i", cc, cc)
        f1a[g, :n] = f1[a:bb]

        a2, bb2 = e2[g], e2[g + 1]
        m = n2[g]
        tcd = np.zeros((TPAD, 3), np.float32)
        tcd[:m] = c2[a2:bb2]
        c2t[g, :3] = tcd.T.reshape(3, TCH, 128)
        c2t[g, 3] = 1.0
        cn = np.einsum("ij,ij->i", tcd, tcd)
        cm[g, :, :, 0] = cn.reshape(TCH, 128).T
        cm[g, :, :, 1] = (np.arange(TPAD) < m).astype(np.float32).reshape(TCH, 128).T
        ff = np.zeros((TPAD, D), np.float32)
        ff[:m] = f2[a2:bb2]
        f2p[g] = ff.reshape(TCH, 128, D).transpose(1, 0, 2)

    f1a_r = f1a.reshape(B, SCH, 128, D).transpose(0, 2, 1, 3)

    in_maps = []
    for c in range(NCORES):
        sl = slice(c * GPC, (c + 1) * GPC)
        in_maps.append(
            {
                "c1r": np.ascontiguousarray(c1r[sl]),
                "f1a": np.ascontiguousarray(f1a_r[sl]),
                "c2t": np.ascontiguousarray(c2t[sl]),
                "cm": np.ascontiguousarray(cm[sl]),
                "f2": np.ascontiguousarray(f2p[sl]),
            }
        )
    return in_maps


_NC_CACHE = None


def _get_nc():
    global _NC_CACHE
    if _NC_CACHE is None:
        _NC_CACHE = _build_nc()
    return _NC_CACHE


def run(inputs, trace=False):
    """Returns (mse_scalar_f32, exec_time_ns_or_None)."""
    from concourse.bass_utils import run_bass_kernel_spmd

    in_maps = _prep_in_maps(inputs)
    nc = _get_nc()
    res = run_bass_kernel_spmd(
        nc, in_maps, core_ids=list(range(NCORES)), trace=trace
    )
    total = 0.0
    for r in res.results:
        total += np.asarray(r["out_sums"], dtype=np.float64).sum()
    mse = np.float32(total / (N * D))
    return mse, res.exec_time_ns


def kernel(**inputs):
    out, _ = run(inputs, trace=False)
    return out


# revision 11
# speedup vs baseline: 1.3305x; 1.3305x over previous
"""Bass/Trainium2 kernel for batched kNN-interpolate + MSE (nn_KnnMSE).

Reference computation (see problem):
  d2[i,j] = ||c2_i - c1_j||^2, masked to same-graph pairs (b1/b2 sorted),
  top-k=8 smallest per target row, w = 1/clip(d2, 1e-16),
  interp = sum(w * f1[idx]) / sum(w),  out = mean((interp - f2)^2).

Strategy: b1/b2 are SORTED graph ids => the valid distance matrix is
block-diagonal over the 64 graphs.  Each graph is an independent
[~256 targets, ~256 sources] kNN problem.  We pad each graph to a fixed
shape (S=320 source slots, 3x128 target rows), assign 8 graphs to each
of the 8 NeuronCores, and on-device per 128-target chunk:

  1. PE    : dist matmul  psum = 2*c2.c1 - ||c1||^2   (K=4 augmented)
  2. ACT   : d2 = relu(-psum + ||c2||^2)              (clamped >= 0)
  3. DVE   : rec = 1/d2  (d2 >= 2.7e-4 for this data; never 0)
  4. DVE   : top8 = max8(rec); thresh = top8[:,7]  (8 nearest = largest rec)
  5. DVE   : W = (rec >= thresh) * rec, sumw = rowsum(W)   [one fused op]
  6. PE    : Wt = W^T (3x 128x128 transposes via identity)
  7. PE    : numer = Wt^T.T @ f1  (contract over sources, PSUM-accumulated)
  8. DVE   : err = numer * (mask/sumw) - f2     (padded rows -> 0)
  9. ACT   : acc[:, chunk] = sum_d(err^2)
  finally : per-partition totals -> DRAM; host sums 8x128 values / (N*D).

Padded sources get coords=1e4 (d2 ~ 3e8, never in top-8, weight->0 via
mask).  Padded targets get coords=0 (finite everywhere) and are zeroed
in step 8 by the mask folded into 1/sumw.

Self-contained: hardcodes shapes from the problem spec; computes graph
boundaries from the actual b1/b2 at call time (host-side index prep only).
"""

import numpy as np

# Problem constants
N = 16384
D = 128
B = 64
KNN = 8
NCORES = 8
GPC = B // NCORES        # graphs per core
S = 320                  # padded source slots per graph (max count 292)
SCH = 3                  # 128-row source chunks covering S (ceil(320/128))
SPAD = SCH * 128         # 384 source slots in the chunked layout
TCH = 3                  # 128-row target chunks per graph (max count 292)
TPAD = TCH * 128
BIGC = 1.0e4             # padded source coordinate


# Perf/precision toggles
import os as _os

# float32r (single-pass fp32 PE matmul) hangs the exec unit with K=4
# stationary tiles (NRT_EXEC_UNIT_UNRECOVERABLE) — keep off.
MM1_F32R = _os.environ.get("KNN_MM1_F32R", "0") == "1"
RECIP_FAST = _os.environ.get("KNN_RECIP_FAST", "1") == "1"  # approx reciprocal
BF16_W = _os.environ.get("KNN_BF16_W", "1") == "1"       # bf16 W/transpose/interp


def _build_nc():
    import concourse.bacc as bacc
    import concourse.mybir as mybir
    import concourse.tile as tile
    from concourse.masks import make_identity

    f32 = mybir.dt.float32
    fmm = mybir.dt.float32r if MM1_F32R else f32
    fw = mybir.dt.bfloat16 if BF16_W else f32
    AF = mybir.ActivationFunctionType
    OP = mybir.AluOpType

    nc = bacc.Bacc("TRN2", target_bir_lowering=False, debug=False)

    c1r_d = nc.dram_tensor("c1r", [GPC, 4, S], fmm, kind="ExternalInput")
    f1a_d = nc.dram_tensor("f1a", [GPC, 128, SCH, D], fw, kind="ExternalInput")
    c2t_d = nc.dram_tensor("c2t", [GPC, 4, TCH, 128], fmm, kind="ExternalInput")
    cm_d = nc.dram_tensor("cm", [GPC, 128, TCH, 2], f32, kind="ExternalInput")
    f2_d = nc.dram_tensor("f2", [GPC, 128, TCH, D], f32, kind="ExternalInput")
    out_d = nc.dram_tensor("out_sums", [128, 1], f32, kind="ExternalOutput")

    with tile.TileContext(nc) as tc:
        with (
            tc.tile_pool(name="constp", bufs=1) as constp,
            tc.tile_pool(name="gbuf", bufs=2) as gbuf,
            tc.tile_pool(name="work", bufs=3) as work,
            tc.tile_pool(name="small", bufs=4) as small,
            tc.tile_pool(name="pdp", bufs=2, space="PSUM") as pdp,
            tc.tile_pool(name="ptp", bufs=3, space="PSUM") as ptp,
            tc.tile_pool(name="pip", bufs=2, space="PSUM") as pip_,
        ):
            ident = constp.tile([128, 128], fw)
            make_identity(nc, ident)
            acc = constp.tile([128, GPC * TCH], f32)
            nc.vector.memset(acc, 0.0)

            for g in range(GPC):
                c1r_t = gbuf.tile([4, S], fmm, tag="c1r")
                nc.sync.dma_start(c1r_t, c1r_d[g])
                f1a_t = gbuf.tile([128, SCH, D], fw, tag="f1a")
                nc.sync.dma_start(f1a_t, f1a_d[g])
                c2t_t = gbuf.tile([4, TCH, 128], fmm, tag="c2t")
                nc.sync.dma_start(c2t_t, c2t_d[g])
                cm_t = gbuf.tile([128, TCH, 2], f32, tag="cm")
                nc.sync.dma_start(cm_t, cm_d[g])
                f2_t = gbuf.tile([128, TCH, D], f32, tag="f2")
                nc.sync.dma_start(f2_t, f2_d[g])

                for t in range(TCH):
                    ci = g * TCH + t
                    # 1) distances: psum = 2*c2.c1 - ||c1||^2  [128 targets, S]
                    pd = pdp.tile([128, S], f32, tag="pd")
                    nc.tensor.matmul(pd, c2t_t[:, t], c1r_t, start=True, stop=True)
                    # 2) d2 = relu(-psum + ||c2||^2)
                    d2 = work.tile([128, S], f32, tag="d2")
                    nc.scalar.activation(
                        d2, pd, AF.Relu, bias=cm_t[:, t, 0:1], scale=-1.0
                    )
                    # 3) rec = 1/d2
                    rec = work.tile([128, S], f32, tag="rec")
                    if RECIP_FAST:
                        nc.vector.reciprocal_approx_fast(out=rec, in_=d2)
                    else:
                        nc.vector.reciprocal(rec, d2)
                    # 4) 8 largest reciprocals per row (= 8 nearest)
                    top8 = small.tile([128, 8], f32, tag="top8")
                    nc.vector.max(out=top8, in_=rec)
                    # 5) W = (rec >= thresh) * rec ; sumw = rowsum(W)
                    W = work.tile([128, S], fw, tag="W")
                    sumw = small.tile([128, 1], f32, tag="sumw")
                    nc.vector.scalar_tensor_tensor(
                        out=W,
                        in0=rec,
                        scalar=top8[:, 7:8],
                        in1=rec,
                        op0=OP.is_ge,
                        op1=OP.mult,
                        accum_out=sumw,
                    )
                    # 6) Wt = W^T (PE transpose per 128-col block)
                    wt = work.tile([128, SCH, 128], fw, tag="wt")
                    for k in range(SCH):
                        w0 = 128 * k
                        cw = min(S, w0 + 128) - w0
                        pt = ptp.tile([128, 128], fw, tag="pt")
                        nc.tensor.transpose(pt[:cw], W[:, w0 : w0 + cw], ident)
                        nc.scalar.copy(wt[:cw, k], pt[:cw])
                        if cw < 128:
                            # stale SBUF here would feed the PE matmul
                            # (NaN * 0 = NaN); zero the unused rows
                            nc.gpsimd.memset(wt[cw:, k], 0.0)
                    # rsw = mask / sumw
                    rsw = small.tile([128, 1], f32, tag="rsw")
                    nc.vector.reciprocal(rsw, sumw)
                    rswm = small.tile([128, 1], f32, tag="rswm")
                    nc.vector.tensor_scalar(
                        rswm, rsw, cm_t[:, t, 1:2], None, op0=OP.mult
                    )
                    # 7) numer = sum_k Wt[:,k]^T @ f1a[:,k]   [128 targets, D]
                    pi = pip_.tile([128, D], f32, tag="pi")
                    for k in range(SCH):
                        nc.tensor.matmul(
                            pi,
                            wt[:, k],
                            f1a_t[:, k],
                            start=(k == 0),
                            stop=(k == SCH - 1),
                        )
                    # 8) err = numer * rswm - f2  (padded target rows -> 0)
                    err = work.tile([128, D], f32, tag="err")
                    nc.vector.scalar_tensor_tensor(
                        out=err,
                        in0=pi,
                        scalar=rswm,
                        in1=f2_t[:, t],
                        op0=OP.mult,
                        op1=OP.subtract,
                    )
                    # 9) acc[:, ci] = sum_d err^2
                    sq = work.tile([128, D], f32, tag="sq")
                    nc.scalar.activation(
                        sq, err, AF.Square, accum_out=acc[:, ci : ci + 1]
                    )

            tot = constp.tile([128, 1], f32)
            nc.vector.reduce_sum(tot, acc, axis=mybir.AxisListType.X)
            nc.sync.dma_start(out_d[:, :], tot)

    nc.compile()
    return nc


def _prep_in_maps(inputs):
    x1 = np.ascontiguousarray(np.asarray(inputs["x1"], dtype=np.float32))
    x2 = np.ascontiguousarray(np.asarray(inputs["x2"], dtype=np.float32))
    b1 = np.asarray(inputs["b1"]).astype(np.int64)
    b2 = np.asarray(inputs["b2"]).astype(np.int64)

    c1, f1 = x1[:, :3], x1[:, 3:]
    c2, f2 = x2[:, :3], x2[:, 3:]

    gs = np.arange(B + 1)
    e1 = np.searchsorted(b1, gs)
    e2 = np.searchsorted(b2, gs)
    n1 = np.diff(e1)
    n2 = np.diff(e2)
    assert n1.max() <= S, f"source count {n1.max()} exceeds padded S={S}"
    assert n2.max() <= TPAD, f"target count {n2.max()} exceeds padded T={TPAD}"
    assert n1.min() >= KNN, f"graph with fewer than {KNN} sources"

    c1r = np.empty((B, 4, S), np.float32)
    f1a = np.zeros((B, SPAD, D), np.float32)
    c2t = np.zeros((B, 4, TCH, 128), np.float32)
    cm = np.zeros((B, 128, TCH, 2), np.float32)
    f2p = np.zeros((B, 128, TCH, D), np.float32)

    for g in range(B):
        a, bb = e1[g], e1[g + 1]
        n = n1[g]
        cc = np.full((S, 3), BIGC, np.float32)
        cc[:n] = c1[a:bb]
        c1r[g, :3] = 2.0 * cc.T
        c1r[g, 3] = -np.einsum("ij,ij->i", cc, cc)
        f1a[g, :n] = f1[a:bb]

        a2, bb2 = e2[g], e2[g + 1]
        m = n2[g]
        tcd = np.zeros((TPAD, 3), np.float32)
        tcd[:m] = c2[a2:bb2]
        c2t[g, :3] = tcd.T.reshape(3, TCH, 128)
        c2t[g, 3] = 1.0
        cn = np.einsum("ij,ij->i", tcd, tcd)
        cm[g, :, :, 0] = cn.reshape(TCH, 128).T
        cm[g, :, :, 1] = (np.arange(TPAD) < m).astype(np.float32).reshape(TCH, 128).T
        ff = np.zeros((TPAD, D), np.float32)
        ff[:m] = f2[a2:bb2]
        f2p[g] = ff.reshape(TCH, 128, D).transpose(1, 0, 2)

    f1a_r = f1a.reshape(B, SCH, 128, D).transpose(0, 2, 1, 3)
    if BF16_W:
        import ml_dtypes

        f1a_r = f1a_r.astype(ml_dtypes.bfloat16)

    in_maps = []
    for c in range(NCORES):
        sl = slice(c * GPC, (c + 1) * GPC)
        in_maps.append(
            {
                "c1r": np.ascontiguousarray(c1r[sl]),
                "f1a": np.ascontiguousarray(f1a_r[sl]),
                "c2t": np.ascontiguousarray(c2t[sl]),
                "cm": np.ascontiguousarray(cm[sl]),
                "f2": np.ascontiguousarray(f2p[sl]),
            }
        )
    return in_maps


_NC_CACHE = None


def _get_nc():
    global _NC_CACHE
    if _NC_CACHE is None:
        _NC_CACHE = _build_nc()
    return _NC_CACHE


def run(inputs, trace=False):
    """Returns (mse_scalar_f32, exec_time_ns_or_None)."""
    from concourse.bass_utils import run_bass_kernel_spmd

    in_maps = _prep_in_maps(inputs)
    nc = _get_nc()
    res = run_bass_kernel_spmd(
        nc, in_maps, core_ids=list(range(NCORES)), trace=trace
    )
    total = 0.0
    for r in res.results:
        total += np.asarray(r["out_sums"], dtype=np.float64).sum()
    mse = np.float32(total / (N * D))
    return mse, res.exec_time_ns


def kernel(**inputs):
    out, _ = run(inputs, trace=False)
    return out


# revision 13
# speedup vs baseline: 1.4876x; 1.1181x over previous
"""Bass/Trainium2 kernel for batched kNN-interpolate + MSE (nn_KnnMSE).

Reference computation (see problem):
  d2[i,j] = ||c2_i - c1_j||^2, masked to same-graph pairs (b1/b2 sorted),
  top-k=8 smallest per target row, w = 1/clip(d2, 1e-16),
  interp = sum(w * f1[idx]) / sum(w),  out = mean((interp - f2)^2).

Strategy: b1/b2 are SORTED graph ids => the valid distance matrix is
block-diagonal over the 64 graphs.  Each graph is an independent
[~256 targets, ~256 sources] kNN problem.  We pad each graph to a fixed
shape (S=320 source slots, 3x128 target rows), assign 8 graphs to each
of the 8 NeuronCores, and on-device per 128-target chunk:

  1. PE    : dist matmul  psum = 2*c2.c1 - ||c1||^2
             (fp16 hi/lo-split, K=11: full ~fp32 precision, single pass)
  2. ACT   : d2 = relu(-psum + ||c2||^2)              (clamped >= 0)
  3. DVE   : rec ~= 1/d2 (custom approx op; d2 >= 2.7e-4 here, never 0)
  4. DVE   : top8 = max8(rec); thresh = top8[:,7]  (8 nearest = largest rec)
  5. DVE   : W = (rec >= thresh) * rec (bf16), sumw = rowsum(W)  [fused]
  6. PE    : Wt = W^T (3x 128x128 bf16 transposes into one PSUM tile)
     ACT   : one batched PSUM->SBUF copy of Wt
  7. PE    : numer = Wt.T @ f1 (bf16, contract over sources, PSUM-accum)
  8. DVE   : err = numer * (1/sumw) - f2   (f2 padded rows are 0)
  9. ACT   : acc[:, chunk] = sum_d (err * mask)^2   (mask kills padded rows)
  finally : per-partition totals -> DRAM; host sums 8x128 values / (N*D).

Padded sources get coords=BIGC (d2 ~ 3e4 >> real d2, never in top-8);
padded targets get coords=0 (finite everywhere) and die at step 9.

Self-contained: hardcodes shapes from the problem spec; computes graph
boundaries from the actual b1/b2 at call time (host-side index prep only).
"""

import os as _os

import numpy as np

# Problem constants
N = 16384
D = 128
B = 64
KNN = 8
NCORES = 8
GPC = B // NCORES        # graphs per core
S = 320                  # padded source slots per graph (max count 292)
SCH = 3                  # 128-row source chunks covering S (ceil(320/128))
SPAD = SCH * 128         # 384 source slots in the chunked layout
TCH = 3                  # 128-row target chunks per graph (max count 292)
TPAD = TCH * 128
# Padded source coordinate.  Must give d2 >> any real within-graph d2
# (~50) while ||c||^2 = 3e4 stays well inside fp16 range (the distance
# matmul runs in fp16 hi/lo-split form).
BIGC = 100.0
KMM = 11                 # dist-matmul contraction: 3x3 hi/lo cross terms + 2 norm rows

RECIP_FAST = _os.environ.get("KNN_RECIP_FAST", "1") == "1"  # approx reciprocal


def _build_nc():
    import concourse.bacc as bacc
    import concourse.mybir as mybir
    import concourse.tile as tile
    from concourse.masks import make_identity

    f32 = mybir.dt.float32
    f16 = mybir.dt.float16
    bf16 = mybir.dt.bfloat16
    AF = mybir.ActivationFunctionType
    OP = mybir.AluOpType

    nc = bacc.Bacc("TRN2", target_bir_lowering=False, debug=False)

    c1r_d = nc.dram_tensor("c1r", [GPC, KMM, S], f16, kind="ExternalInput")
    f1a_d = nc.dram_tensor("f1a", [GPC, 128, SCH, D], bf16, kind="ExternalInput")
    c2t_d = nc.dram_tensor("c2t", [GPC, KMM, TCH, 128], f16, kind="ExternalInput")
    cm_d = nc.dram_tensor("cm", [GPC, 128, TCH, 2], f32, kind="ExternalInput")
    f2_d = nc.dram_tensor("f2", [GPC, 128, TCH, D], f32, kind="ExternalInput")
    out_d = nc.dram_tensor("out_sums", [128, 1], f32, kind="ExternalOutput")

    with tile.TileContext(nc) as tc:
        with (
            tc.tile_pool(name="constp", bufs=1) as constp,
            tc.tile_pool(name="gbuf", bufs=3) as gbuf,
            tc.tile_pool(name="work", bufs=4) as work,
            tc.tile_pool(name="small", bufs=6) as small,
            tc.tile_pool(name="pdp", bufs=3, space="PSUM") as pdp,
            tc.tile_pool(name="ptp", bufs=2, space="PSUM") as ptp,
            tc.tile_pool(name="pip", bufs=2, space="PSUM") as pip_,
        ):
            ident = constp.tile([128, 128], bf16)
            make_identity(nc, ident)
            acc = constp.tile([128, GPC * TCH], f32)
            nc.vector.memset(acc, 0.0)

            for g in range(GPC):
                c1r_t = gbuf.tile([KMM, S], f16, tag="c1r")
                nc.sync.dma_start(c1r_t, c1r_d[g])
                f1a_t = gbuf.tile([128, SCH, D], bf16, tag="f1a")
                nc.sync.dma_start(f1a_t, f1a_d[g])
                c2t_t = gbuf.tile([KMM, TCH, 128], f16, tag="c2t")
                nc.sync.dma_start(c2t_t, c2t_d[g])
                cm_t = gbuf.tile([128, TCH, 2], f32, tag="cm")
                nc.sync.dma_start(cm_t, cm_d[g])
                f2_t = gbuf.tile([128, TCH, D], f32, tag="f2")
                nc.sync.dma_start(f2_t, f2_d[g])

                for t in range(TCH):
                    ci = g * TCH + t
                    # 1) distances: psum = 2*c2.c1 - ||c1||^2  [128 targets, S]
                    pd = pdp.tile([128, S], f32, tag="pd")
                    nc.tensor.matmul(pd, c2t_t[:, t], c1r_t, start=True, stop=True)
                    # 2) d2 = relu(-psum + ||c2||^2)
                    d2 = work.tile([128, S], f32, tag="d2")
                    nc.scalar.activation(
                        d2, pd, AF.Relu, bias=cm_t[:, t, 0:1], scale=-1.0
                    )
                    # 3) rec = 1/d2
                    rec = work.tile([128, S], f32, tag="rec")
                    if RECIP_FAST:
                        nc.vector.reciprocal_approx_fast(out=rec, in_=d2)
                    else:
                        nc.vector.reciprocal(rec, d2)
                    # 4) 8 largest reciprocals per row (= 8 nearest)
                    top8 = small.tile([128, 8], f32, tag="top8")
                    nc.vector.max(out=top8, in_=rec)
                    # 5) W = (rec >= thresh) * rec ; sumw = rowsum(W)
                    W = work.tile([128, S], bf16, tag="W")
                    sumw = small.tile([128, 1], f32, tag="sumw")
                    nc.vector.scalar_tensor_tensor(
                        out=W,
                        in0=rec,
                        scalar=top8[:, 7:8],
                        in1=rec,
                        op0=OP.is_ge,
                        op1=OP.mult,
                        accum_out=sumw,
                    )
                    # 6) Wt = W^T: 3 PE transposes into ONE psum tile, then
                    #    one batched ACT copy to SBUF.  The 64-row tail of
                    #    chunk 2 stays garbage; step 7 reads only [:64].
                    pt = ptp.tile([128, SCH, 128], bf16, tag="pt")
                    for k in range(SCH):
                        w0 = 128 * k
                        cw = min(S, w0 + 128) - w0
                        nc.tensor.transpose(pt[:cw, k], W[:, w0 : w0 + cw], ident)
                    wt = work.tile([128, SCH, 128], bf16, tag="wt")
                    nc.scalar.copy(wt, pt)
                    # rsw = 1/sumw
                    rsw = small.tile([128, 1], f32, tag="rsw")
                    nc.vector.reciprocal(rsw, sumw)
                    # 7) numer = sum_k Wt[:,k]^T @ f1a[:,k]   [128 targets, D]
                    pi = pip_.tile([128, D], f32, tag="pi")
                    for k in range(SCH):
                        w0 = 128 * k
                        cw = min(S, w0 + 128) - w0
                        nc.tensor.matmul(
                            pi,
                            wt[:cw, k],
                            f1a_t[:cw, k],
                            start=(k == 0),
                            stop=(k == SCH - 1),
                        )
                    # 8) err = numer * rsw - f2  (f2 padded rows are 0)
                    err = work.tile([128, D], f32, tag="err")
                    nc.vector.scalar_tensor_tensor(
                        out=err,
                        in0=pi,
                        scalar=rsw,
                        in1=f2_t[:, t],
                        op0=OP.mult,
                        op1=OP.subtract,
                    )
                    # 9) acc[:, ci] = sum_d (err*mask)^2
                    sq = work.tile([128, D], f32, tag="sq")
                    nc.scalar.activation(
                        sq,
                        err,
                        AF.Square,
                        scale=cm_t[:, t, 1:2],
                        accum_out=acc[:, ci : ci + 1],
                    )

            tot = constp.tile([128, 1], f32)
            nc.vector.reduce_sum(tot, acc, axis=mybir.AxisListType.X)
            nc.sync.dma_start(out_d[:, :], tot)

    nc.compile()
    return nc


def _hl(x):
    """fp16 hi/lo split: x ~= hi + lo with both parts exact in fp16."""
    hi = x.astype(np.float16)
    lo = (x - hi.astype(np.float32)).astype(np.float16)
    return hi, lo


def _prep_in_maps(inputs):
    import ml_dtypes

    x1 = np.ascontiguousarray(np.asarray(inputs["x1"], dtype=np.float32))
    x2 = np.ascontiguousarray(np.asarray(inputs["x2"], dtype=np.float32))
    b1 = np.asarray(inputs["b1"]).astype(np.int64)
    b2 = np.asarray(inputs["b2"]).astype(np.int64)

    c1, f1 = x1[:, :3], x1[:, 3:]
    c2, f2 = x2[:, :3], x2[:, 3:]

    gs = np.arange(B + 1)
    e1 = np.searchsorted(b1, gs)
    e2 = np.searchsorted(b2, gs)
    n1 = np.diff(e1)
    n2 = np.diff(e2)
    assert n1.max() <= S, f"source count {n1.max()} exceeds padded S={S}"
    assert n2.max() <= TPAD, f"target count {n2.max()} exceeds padded T={TPAD}"
    assert n1.min() >= KNN, f"graph with fewer than {KNN} sources"

    c1r = np.zeros((B, KMM, S), np.float16)
    f1a = np.zeros((B, SPAD, D), np.float32)
    c2t = np.zeros((B, KMM, TPAD), np.float16)
    cm = np.zeros((B, 128, TCH, 2), np.float32)
    f2p = np.zeros((B, 128, TCH, D), np.float32)

    for g in range(B):
        a, bb = e1[g], e1[g + 1]
        n = n1[g]
        cc = np.full((S, 3), BIGC, np.float32)
        cc[:n] = c1[a:bb]
        h1, l1 = _hl(cc)  # [S, 3] each
        # rhs rows paired with lhsT rows (see c2t below):
        #  0-2: 2*hi1   (x hi2)    3-5: 2*lo1 (x hi2)    6-8: 2*hi1 (x lo2)
        #  9: -n_hi (x 1)          10: -n_lo (x 1)
        c1r[g, 0:3] = (2.0 * h1.astype(np.float32)).astype(np.float16).T
        c1r[g, 3:6] = (2.0 * l1.astype(np.float32)).astype(np.float16).T
        c1r[g, 6:9] = c1r[g, 0:3]
        nrm = np.einsum("ij,ij->i", cc, cc)
        nh, nl = _hl(nrm)
        c1r[g, 9] = -nh
        c1r[g, 10] = -nl
        f1a[g, :n] = f1[a:bb]

        a2, bb2 = e2[g], e2[g + 1]
        m = n2[g]
        tcd = np.zeros((TPAD, 3), np.float32)
        tcd[:m] = c2[a2:bb2]
        h2, l2 = _hl(tcd)  # [TPAD, 3]
        c2t[g, 0:3] = h2.T
        c2t[g, 3:6] = h2.T
        c2t[g, 6:9] = l2.T
        c2t[g, 9:11] = 1.0
        cn = np.einsum("ij,ij->i", tcd, tcd)
        cm[g, :, :, 0] = cn.reshape(TCH, 128).T
        cm[g, :, :, 1] = (np.arange(TPAD) < m).astype(np.float32).reshape(TCH, 128).T
        ff = np.zeros((TPAD, D), np.float32)
        ff[:m] = f2[a2:bb2]
        f2p[g] = ff.reshape(TCH, 128, D).transpose(1, 0, 2)

    c2t_r = c2t.reshape(B, KMM, TCH, 128)
    f1a_r = f1a.reshape(B, SCH, 128, D).transpose(0, 2, 1, 3).astype(ml_dtypes.bfloat16)

    in_maps = []
    for c in range(NCORES):
        sl = slice(c * GPC, (c + 1) * GPC)
        in_maps.append(
            {
                "c1r": np.ascontiguousarray(c1r[sl]),
                "f1a": np.ascontiguousarray(f1a_r[sl]),
                "c2t": np.ascontiguousarray(c2t_r[sl]),
                "cm": np.ascontiguousarray(cm[sl]),
                "f2": np.ascontiguousarray(f2p[sl]),
            }
        )
    return in_maps


_NC_CACHE = None


def _get_nc():
    global _NC_CACHE
    if _NC_CACHE is None:
        _NC_CACHE = _build_nc()
    return _NC_CACHE


def run(inputs, trace=False):
    """Returns (mse_scalar_f32, exec_time_ns_or_None)."""
    from concourse.bass_utils import run_bass_kernel_spmd

    in_maps = _prep_in_maps(inputs)
    nc = _get_nc()
    res = run_bass_kernel_spmd(
        nc, in_maps, core_ids=list(range(NCORES)), trace=trace
    )
    total = 0.0
    for r in res.results:
        total += np.asarray(r["out_sums"], dtype=np.float64).sum()
    mse = np.float32(total / (N * D))
    return mse, res.exec_time_ns


def kernel(**inputs):
    out, _ = run(inputs, trace=False)
    return out
